# revision 2
# baseline (speedup 1.0000x reference)
import sys

import numpy as np

for p in ("/opt/trn_rl_repo",):
    if p not in sys.path:
        sys.path.insert(0, p)

import concourse.bass as bass  # noqa: E402
import concourse.tile as tile  # noqa: E402
from concourse import bacc, mybir  # noqa: E402
from concourse.bass_utils import run_bass_kernel_spmd  # noqa: E402

B, N, D = 128, 512, 512
NCORES = 8
BPC = B // NCORES  # 16 batch items per core
F32 = mybir.dt.float32


def _hadamard(n: int) -> np.ndarray:
    H = np.array([[1.0]], dtype=np.float32)
    base = np.array([[1.0, 1.0], [1.0, -1.0]], dtype=np.float32)
    while H.shape[0] < n:
        H = np.kron(H, base)
    return H


def _build():
    nc = bacc.Bacc("TRN2", target_bir_lowering=False, debug=False)
    x_d = nc.dram_tensor("x", [BPC, N, D], F32, kind="ExternalInput").ap()
    h_d = nc.dram_tensor("h", [128, 4 * N], F32, kind="ExternalInput").ap()
    hs_d = nc.dram_tensor("hs", [128, 4 * N], F32, kind="ExternalInput").ap()
    y_d = nc.dram_tensor("y", [BPC, N, D], F32, kind="ExternalOutput").ap()

    with tile.TileContext(nc) as tc:
        with (
            tc.tile_pool(name="const", bufs=1) as const_pool,
            tc.tile_pool(name="xp", bufs=3) as x_pool,
            tc.tile_pool(name="tp", bufs=2) as t_pool,
            tc.tile_pool(name="op", bufs=8) as o_pool,
            tc.tile_pool(name="ps", bufs=8, space="PSUM") as psum_pool,
        ):
            # H laid out as [128, c*512+n] with row m = c*128+p
            h_sb = const_pool.tile([128, 4 * N], F32)
            nc.sync.dma_start(h_sb[:], h_d[:])
            hs_sb = const_pool.tile([128, 4 * N], F32, tag="hs")
            nc.sync.dma_start(hs_sb[:], hs_d[:])

            for b in range(BPC):
                xt = x_pool.tile([128, 4 * D], F32)
                for c in range(4):
                    nc.sync.dma_start(
                        xt[:, c * D : (c + 1) * D],
                        x_d[b, c * 128 : (c + 1) * 128, :],
                    )
                # t_T[d, n] = sum_m x[m, d] * H[m, n]  (H symmetric)
                tt = t_pool.tile([128, 4 * N], F32)
                for dt_ in range(4):
                    ps = psum_pool.tile([128, N], F32)
                    for kc in range(4):
                        nc.tensor.matmul(
                            ps[:],
                            xt[:, kc * D + dt_ * 128 : kc * D + dt_ * 128 + 128],
                            h_sb[:, kc * N : (kc + 1) * N],
                            start=(kc == 0),
                            stop=(kc == 3),
                        )
                    nc.any.tensor_copy(tt[:, dt_ * N : (dt_ + 1) * N], ps[:])
                # y[n, e] = sum_d t_T[d, n] * (H/512)[d, e]
                for nt in range(4):
                    ps = psum_pool.tile([128, D], F32)
                    for dc in range(4):
                        nc.tensor.matmul(
                            ps[:],
                            tt[:, dc * N + nt * 128 : dc * N + nt * 128 + 128],
                            hs_sb[:, dc * D : (dc + 1) * D],
                            start=(dc == 0),
                            stop=(dc == 3),
                        )
                    ot = o_pool.tile([128, D], F32)
                    nc.any.tensor_copy(ot[:], ps[:])
                    nc.sync.dma_start(y_d[b, nt * 128 : (nt + 1) * 128, :], ot[:])

    nc.compile()
    return nc


_NC = None


def _get_nc():
    global _NC
    if _NC is None:
        _NC = _build()
    return _NC


def _in_maps(x: np.ndarray) -> list:
    x = np.ascontiguousarray(np.asarray(x), dtype=np.float32)
    H = _hadamard(N)
    # layout [128, c*512+n] with row m = c*128+p
    h_l = np.ascontiguousarray(
        H.reshape(4, 128, N).transpose(1, 0, 2).reshape(128, 4 * N)
    )
    hs_l = np.ascontiguousarray(h_l / np.float32(512.0))
    return [
        {"x": x[i * BPC : (i + 1) * BPC], "h": h_l, "hs": hs_l}
        for i in range(NCORES)
    ]


def kernel(x: np.ndarray) -> np.ndarray:
    nc = _get_nc()
    res = run_bass_kernel_spmd(nc, _in_maps(x), list(range(NCORES))).results
    return np.concatenate([r["y"] for r in res], axis=0).astype(np.float32)



# revision 3
# speedup vs baseline: 3.4013x; 3.4013x over previous
import sys

import numpy as np

for p in ("/opt/trn_rl_repo",):
    if p not in sys.path:
        sys.path.insert(0, p)

import concourse.bass as bass  # noqa: E402
import concourse.tile as tile  # noqa: E402
from concourse import bacc, mybir  # noqa: E402
from concourse.bass_utils import run_bass_kernel_spmd  # noqa: E402

B, N, D = 128, 512, 512
NCORES = 8
BPC = B // NCORES  # 16 batch items per core
F32 = mybir.dt.float32
BF16 = mybir.dt.bfloat16


def _hadamard(n: int) -> np.ndarray:
    H = np.array([[1.0]], dtype=np.float32)
    base = np.array([[1.0, 1.0], [1.0, -1.0]], dtype=np.float32)
    while H.shape[0] < n:
        H = np.kron(H, base)
    return H


def _build():
    nc = bacc.Bacc("TRN2", target_bir_lowering=False, debug=False)
    x_d = nc.dram_tensor("x", [BPC, N, D], F32, kind="ExternalInput").ap()
    # h[q, c*512+n] = H[c*128+q, n]; hs same but scaled by 1/512
    h_d = nc.dram_tensor("h", [128, 4 * N], BF16, kind="ExternalInput").ap()
    hs_d = nc.dram_tensor("hs", [128, 4 * N], BF16, kind="ExternalInput").ap()
    y_d = nc.dram_tensor("y", [BPC, N, D], BF16, kind="ExternalOutput").ap()

    with tile.TileContext(nc) as tc:
        with (
            tc.tile_pool(name="const", bufs=1) as const_pool,
            tc.tile_pool(name="xp", bufs=3) as x_pool,
            tc.tile_pool(name="xb", bufs=3) as xb_pool,
            tc.tile_pool(name="tp", bufs=2) as t_pool,
            tc.tile_pool(name="op", bufs=8) as o_pool,
            tc.tile_pool(name="psa", bufs=4, space="PSUM") as psum_a,
            tc.tile_pool(name="psb", bufs=4, space="PSUM") as psum_b,
        ):
            h_sb = const_pool.tile([128, 4 * N], BF16)
            nc.sync.dma_start(h_sb[:], h_d[:])
            hs_sb = const_pool.tile([128, 4 * N], BF16, tag="hs")
            nc.sync.dma_start(hs_sb[:], hs_d[:])

            for b in range(BPC):
                xt = x_pool.tile([128, 4 * D], F32)
                xb = xb_pool.tile([128, 4 * D], BF16)
                for c in range(4):
                    nc.sync.dma_start(
                        xt[:, c * D : (c + 1) * D],
                        x_d[b, c * 128 : (c + 1) * 128, :],
                    )
                    # fp32 -> bf16 cast, split across scalar and vector
                    eng = nc.scalar if c % 2 == 0 else nc.vector
                    if eng is nc.scalar:
                        eng.copy(
                            xb[:, c * D : (c + 1) * D], xt[:, c * D : (c + 1) * D]
                        )
                    else:
                        eng.tensor_copy(
                            xb[:, c * D : (c + 1) * D], xt[:, c * D : (c + 1) * D]
                        )
                # A: t_T[d, n] = sum_m x[m, d] * H[m, n]
                tt = t_pool.tile([128, 4 * N], BF16)
                for dt_ in range(4):
                    ps = psum_a.tile([128, N], F32)
                    for kc in range(4):
                        nc.tensor.matmul(
                            ps[:],
                            xb[:, kc * D + dt_ * 128 : kc * D + dt_ * 128 + 128],
                            h_sb[:, kc * N : (kc + 1) * N],
                            start=(kc == 0),
                            stop=(kc == 3),
                        )
                    nc.scalar.copy(tt[:, dt_ * N : (dt_ + 1) * N], ps[:])
                # B: y[n, e] = sum_d t_T[d, n] * (H/512)[d, e]
                for nt in range(4):
                    ps = psum_b.tile([128, D], F32)
                    for dc in range(4):
                        nc.tensor.matmul(
                            ps[:],
                            tt[:, dc * N + nt * 128 : dc * N + nt * 128 + 128],
                            hs_sb[:, dc * D : (dc + 1) * D],
                            start=(dc == 0),
                            stop=(dc == 3),
                        )
                    ot = o_pool.tile([128, D], BF16)
                    nc.vector.tensor_copy(ot[:], ps[:])
                    nc.sync.dma_start(y_d[b, nt * 128 : (nt + 1) * 128, :], ot[:])

    nc.compile()
    return nc


_NC = None


def _get_nc():
    global _NC
    if _NC is None:
        _NC = _build()
    return _NC


def _in_maps(x: np.ndarray) -> list:
    import ml_dtypes

    x = np.ascontiguousarray(np.asarray(x), dtype=np.float32)
    H = _hadamard(N)
    h_l = np.ascontiguousarray(
        H.reshape(4, 128, N).transpose(1, 0, 2).reshape(128, 4 * N)
    ).astype(ml_dtypes.bfloat16)
    hs_l = np.ascontiguousarray(
        (H / np.float32(512.0))
        .reshape(4, 128, N)
        .transpose(1, 0, 2)
        .reshape(128, 4 * N)
    ).astype(ml_dtypes.bfloat16)
    return [
        {"x": x[i * BPC : (i + 1) * BPC], "h": h_l, "hs": hs_l}
        for i in range(NCORES)
    ]


def kernel(x: np.ndarray) -> np.ndarray:
    nc = _get_nc()
    res = run_bass_kernel_spmd(nc, _in_maps(x), list(range(NCORES))).results
    return np.concatenate(
        [np.asarray(r["y"]).astype(np.float32) for r in res], axis=0
    )


# revision 7
# speedup vs baseline: 3.4179x; 1.0049x over previous
import sys

import numpy as np

for p in ("/opt/trn_rl_repo",):
    if p not in sys.path:
        sys.path.insert(0, p)

import concourse.bass as bass  # noqa: E402
import concourse.tile as tile  # noqa: E402
from concourse import bacc, mybir  # noqa: E402
from concourse.bass_utils import run_bass_kernel_spmd  # noqa: E402

B, N, D = 128, 512, 512
NCORES = 8
BPC = B // NCORES  # 16 batch items per core
F32 = mybir.dt.float32
BF16 = mybir.dt.bfloat16


def _hadamard(n: int) -> np.ndarray:
    H = np.array([[1.0]], dtype=np.float32)
    base = np.array([[1.0, 1.0], [1.0, -1.0]], dtype=np.float32)
    while H.shape[0] < n:
        H = np.kron(H, base)
    return H


def _build():
    # Layout: batch row m = 4q + j lives on partition q, free-block j
    # (4 consecutive DRAM rows per partition -> 4KB bf16 DMA lines).
    # H512[4q+j, 4p+i] = H128[q,p] * H4[j,i] (Sylvester, H512 = H128 x H4).
    # Transform A (over m): psum accumulates u_j pairs via moving +/-H128
    # (H2 level folded into PSUM); remaining H2 level is a GPSIMD butterfly.
    # Transform B (over d): plain 4-block matmuls with H512/512.
    nc = bacc.Bacc("TRN2", target_bir_lowering=False, debug=False)
    x_d = nc.dram_tensor("x", [BPC, 128, 4 * D], BF16, kind="ExternalInput").ap()
    s2_d = nc.dram_tensor("s2", [128, 256], BF16, kind="ExternalInput").ap()
    hs_d = nc.dram_tensor("hs", [128, 4 * D], BF16, kind="ExternalInput").ap()
    y_d = nc.dram_tensor("y", [BPC, 128, 4 * D], BF16, kind="ExternalOutput").ap()

    with tile.TileContext(nc) as tc:
        with (
            tc.tile_pool(name="const", bufs=1) as const_pool,
            tc.tile_pool(name="xp", bufs=3) as x_pool,
            tc.tile_pool(name="cs", bufs=2) as cs_pool,
            tc.tile_pool(name="tp", bufs=2) as t_pool,
            tc.tile_pool(name="yp", bufs=2) as y_pool,
            tc.tile_pool(name="psa", bufs=4, space="PSUM") as psum_a,
            tc.tile_pool(name="psb", bufs=4, space="PSUM") as psum_b,
        ):
            s2_sb = const_pool.tile([128, 256], BF16)
            nc.sync.dma_start(s2_sb[:], s2_d[:])
            hs_sb = const_pool.tile([128, 4 * D], BF16, tag="hs")

            S = s2_sb[:, 0:128]
            Sn = s2_sb[:, 128:256]

            for b in range(BPC):
                xb = x_pool.tile([128, 4 * D], BF16)
                nc.sync.dma_start(xb[:], x_d[b])
                if b == 0:
                    nc.sync.dma_start(hs_sb[:], hs_d[:])

                # A: u_j[dd, p] = sum_q x[4q+j, dt*128+dd] * H128[q, p]
                # psum free layout dt*128+p; s01=u0+u1, d01=u0-u1, ...
                ps_s01 = psum_a.tile([128, 4 * 128], F32, tag="s01", bufs=1)
                ps_d01 = psum_a.tile([128, 4 * 128], F32, tag="d01", bufs=1)
                ps_s23 = psum_a.tile([128, 4 * 128], F32, tag="s23", bufs=1)
                ps_d23 = psum_a.tile([128, 4 * 128], F32, tag="d23", bufs=1)
                for dt in range(4):
                    o = slice(dt * 128, dt * 128 + 128)
                    x0 = xb[:, 0 * D + dt * 128 : 0 * D + dt * 128 + 128]
                    x1 = xb[:, 1 * D + dt * 128 : 1 * D + dt * 128 + 128]
                    x2 = xb[:, 2 * D + dt * 128 : 2 * D + dt * 128 + 128]
                    x3 = xb[:, 3 * D + dt * 128 : 3 * D + dt * 128 + 128]
                    nc.tensor.matmul(ps_s01[:, o], x0, S, start=True, stop=False)
                    nc.tensor.matmul(ps_d01[:, o], x0, S, start=True, stop=False)
                    nc.tensor.matmul(ps_s01[:, o], x1, S, start=False, stop=True)
                    nc.tensor.matmul(ps_d01[:, o], x1, Sn, start=False, stop=True)
                    nc.tensor.matmul(ps_s23[:, o], x2, S, start=True, stop=False)
                    nc.tensor.matmul(ps_d23[:, o], x2, S, start=True, stop=False)
                    nc.tensor.matmul(ps_s23[:, o], x3, S, start=False, stop=True)
                    nc.tensor.matmul(ps_d23[:, o], x3, Sn, start=False, stop=True)
                cs_s01 = cs_pool.tile([128, 4 * 128], BF16, tag="s01")
                cs_d01 = cs_pool.tile([128, 4 * 128], BF16, tag="d01")
                cs_s23 = cs_pool.tile([128, 4 * 128], BF16, tag="s23")
                cs_d23 = cs_pool.tile([128, 4 * 128], BF16, tag="d23")
                nc.scalar.copy(cs_s01[:], ps_s01[:])
                nc.scalar.copy(cs_d01[:], ps_d01[:])
                nc.scalar.copy(cs_s23[:], ps_s23[:])
                nc.scalar.copy(cs_d23[:], ps_d23[:])
                # H2 butterfly over j on GPSIMD:
                # tt_i[dd, dt*128+p] = t_T[dt*128+dd, 4p+i]
                tt0 = t_pool.tile([128, 4 * 128], BF16, tag="t0")
                tt1 = t_pool.tile([128, 4 * 128], BF16, tag="t1")
                tt2 = t_pool.tile([128, 4 * 128], BF16, tag="t2")
                tt3 = t_pool.tile([128, 4 * 128], BF16, tag="t3")
                nc.gpsimd.tensor_add(tt0[:], cs_s01[:], cs_s23[:])
                nc.gpsimd.tensor_add(tt1[:], cs_d01[:], cs_d23[:])
                nc.gpsimd.tensor_sub(tt2[:], cs_s01[:], cs_s23[:])
                nc.gpsimd.tensor_sub(tt3[:], cs_d01[:], cs_d23[:])
                tts = (tt0, tt1, tt2, tt3)
                # B: y[4p+j, e] = sum_d t_T[d, 4p+j] * (H512/512)[d, e]
                yb = y_pool.tile([128, 4 * D], BF16)
                for j in range(4):
                    ps = psum_b.tile([128, D], F32)
                    for dc in range(4):
                        nc.tensor.matmul(
                            ps[:],
                            tts[j][:, dc * 128 : dc * 128 + 128],
                            hs_sb[:, dc * D : (dc + 1) * D],
                            start=(dc == 0),
                            stop=(dc == 3),
                        )
                    nc.vector.tensor_copy(yb[:, j * D : (j + 1) * D], ps[:])
                nc.sync.dma_start(y_d[b], yb[:])

    nc.compile()
    return nc


_NC = None


def _get_nc():
    global _NC
    if _NC is None:
        _NC = _build()
    return _NC


def _in_maps(x: np.ndarray) -> list:
    import ml_dtypes

    bf16 = ml_dtypes.bfloat16
    x = np.asarray(x)
    xb = np.ascontiguousarray(x, dtype=np.float32).astype(bf16)
    H128 = _hadamard(128)
    s2 = np.ascontiguousarray(
        np.concatenate([H128, -H128], axis=1)
    ).astype(bf16)
    H = _hadamard(N)
    hs = np.ascontiguousarray(
        (H / np.float32(512.0))
        .reshape(4, 128, N)
        .transpose(1, 0, 2)
        .reshape(128, 4 * N)
    ).astype(bf16)
    return [
        {
            "x": np.ascontiguousarray(
                xb[i * BPC : (i + 1) * BPC].reshape(BPC, 128, 4 * D)
            ),
            "s2": s2,
            "hs": hs,
        }
        for i in range(NCORES)
    ]


def kernel(x: np.ndarray) -> np.ndarray:
    nc = _get_nc()
    res = run_bass_kernel_spmd(nc, _in_maps(x), list(range(NCORES))).results
    return np.concatenate(
        [np.asarray(r["y"]).reshape(BPC, N, D).astype(np.float32) for r in res],
        axis=0,
    )


# revision 8
# speedup vs baseline: 4.3368x; 1.2688x over previous
import sys

import numpy as np

for p in ("/opt/trn_rl_repo",):
    if p not in sys.path:
        sys.path.insert(0, p)

import concourse.bass as bass  # noqa: E402
import concourse.tile as tile  # noqa: E402
from concourse import bacc, mybir  # noqa: E402
from concourse.bass_utils import run_bass_kernel_spmd  # noqa: E402

B, N, D = 128, 512, 512
NCORES = 8
BPC = B // NCORES  # 16 batch items per core
F32 = mybir.dt.float32
BF16 = mybir.dt.bfloat16


def _hadamard(n: int) -> np.ndarray:
    H = np.array([[1.0]], dtype=np.float32)
    base = np.array([[1.0, 1.0], [1.0, -1.0]], dtype=np.float32)
    while H.shape[0] < n:
        H = np.kron(H, base)
    return H


def _build():
    # Row m = 4q+j lives on partition q, free block j (4 consecutive DRAM
    # rows per partition -> one 512KB DMA per batch, 4KB lines).
    # H512[4q+j, 4p+i] = H128[q,p] * H4[j,i]  (H512 = H128 (x) H4).
    # Transform A folds one H2 level into PSUM accumulation using paired
    # moving operands [S|S] / [S|-S] (N=256 matmuls); the remaining H2
    # level is two big elementwise ops (DVE + GPSIMD).
    # Transform B is plain 4-block matmuls with H512/512 (N=512).
    nc = bacc.Bacc("TRN2", target_bir_lowering=False, debug=False)
    x_d = nc.dram_tensor("x", [BPC, 128, 4 * D], BF16, kind="ExternalInput").ap()
    s4_d = nc.dram_tensor("s4", [128, 512], BF16, kind="ExternalInput").ap()
    hs_d = nc.dram_tensor("hs", [128, 4 * D], BF16, kind="ExternalInput").ap()
    y_d = nc.dram_tensor("y", [BPC, 128, 4 * D], BF16, kind="ExternalOutput").ap()

    with tile.TileContext(nc) as tc:
        with (
            tc.tile_pool(name="const", bufs=1) as const_pool,
            tc.tile_pool(name="xp", bufs=3) as x_pool,
            tc.tile_pool(name="cs", bufs=2) as cs_pool,
            tc.tile_pool(name="tp", bufs=2) as t_pool,
            tc.tile_pool(name="yp", bufs=2) as y_pool,
            tc.tile_pool(name="psa", bufs=1, space="PSUM") as psum_a,
            tc.tile_pool(name="psb", bufs=4, space="PSUM") as psum_b,
        ):
            s4_sb = const_pool.tile([128, 512], BF16)
            nc.sync.dma_start(s4_sb[:], s4_d[:])
            hs_sb = const_pool.tile([128, 4 * D], BF16, tag="hs")

            sp2 = s4_sb[:, 0:256]  # [S | S]
            sn2 = s4_sb[:, 256:512]  # [S | -S]

            state = None  # pending (tts, yb, b) from previous batch

            def emit_B(tts, yb, bprev):
                # B: y[4p+j, e] = sum_d t_T[d, 4p+j] * (H512/512)[d, e]
                for j in range(4):
                    ps = psum_b.tile([128, D], F32, name="psb")
                    for dc in range(4):
                        nc.tensor.matmul(
                            ps[:],
                            tts[j][:, dc * 128 : dc * 128 + 128],
                            hs_sb[:, dc * D : (dc + 1) * D],
                            start=(dc == 0),
                            stop=(dc == 3),
                        )
                    nc.vector.tensor_copy(yb[:, j * D : (j + 1) * D], ps[:])
                nc.sync.dma_start(y_d[bprev], yb[:])

            for b in range(BPC):
                xb = x_pool.tile([128, 4 * D], BF16)
                nc.sync.dma_start(xb[:], x_d[b])
                if b == 0:
                    nc.sync.dma_start(hs_sb[:], hs_d[:])

                # A: u_j[dd, p] = sum_q x[4q+j, dt*128+dd] * H128[q, p]
                # psum free layout dt*256 + (s:0-127 | d:128-255)
                # sd01 = [u0+u1 | u0-u1],  sd23 = [u2+u3 | u2-u3]
                ps_sd01 = psum_a.tile([128, 1024], F32, tag="sd01")
                ps_sd23 = psum_a.tile([128, 1024], F32, tag="sd23")
                for dt in range(4):
                    o = slice(dt * 256, dt * 256 + 256)
                    x0 = xb[:, 0 * D + dt * 128 : 0 * D + dt * 128 + 128]
                    x1 = xb[:, 1 * D + dt * 128 : 1 * D + dt * 128 + 128]
                    x2 = xb[:, 2 * D + dt * 128 : 2 * D + dt * 128 + 128]
                    x3 = xb[:, 3 * D + dt * 128 : 3 * D + dt * 128 + 128]
                    nc.tensor.matmul(ps_sd01[:, o], x0, sp2, start=True, stop=False)
                    nc.tensor.matmul(ps_sd01[:, o], x1, sn2, start=False, stop=True)
                    nc.tensor.matmul(ps_sd23[:, o], x2, sp2, start=True, stop=False)
                    nc.tensor.matmul(ps_sd23[:, o], x3, sn2, start=False, stop=True)
                cs_sd01 = cs_pool.tile([128, 1024], BF16, tag="sd01")
                cs_sd23 = cs_pool.tile([128, 1024], BF16, tag="sd23")
                nc.scalar.copy(cs_sd01[:], ps_sd01[:])
                nc.scalar.copy(cs_sd23[:], ps_sd23[:])
                # Remaining H2 level: ttP = sd01+sd23 = [t0|t1] chunks,
                # ttM = sd01-sd23 = [t2|t3] chunks (layout dt*256 + i*128+p)
                ttP = t_pool.tile([128, 1024], BF16, tag="P")
                ttM = t_pool.tile([128, 1024], BF16, tag="M")
                nc.vector.tensor_add(ttP[:], cs_sd01[:], cs_sd23[:])
                nc.gpsimd.tensor_sub(ttM[:], cs_sd01[:], cs_sd23[:])
                # lhsT slice views: tts[j][:, dc*128:+128]
                tts = {
                    0: ttP[:].rearrange("p (t s) -> p t s", t=4)[:, :, 0:128],
                    1: ttP[:].rearrange("p (t s) -> p t s", t=4)[:, :, 128:256],
                    2: ttM[:].rearrange("p (t s) -> p t s", t=4)[:, :, 0:128],
                    3: ttM[:].rearrange("p (t s) -> p t s", t=4)[:, :, 128:256],
                }
                # flatten back to [128, 512] style accessor
                tts = {
                    j: TTSView(ap) for j, ap in tts.items()
                }
                yb = y_pool.tile([128, 4 * D], BF16)
                if state is not None:
                    emit_B(*state)
                state = (tts, yb, b)

            emit_B(*state)

    nc.compile()
    return nc


class TTSView:
    """View of a [128, 4, 128] AP exposing [:, dc*128:+128] slicing."""

    def __init__(self, ap):
        self.ap = ap

    def __getitem__(self, idx):
        _, fsl = idx
        dc = fsl.start // 128
        return self.ap[:, dc, :]


_NC = None


def _get_nc():
    global _NC
    if _NC is None:
        _NC = _build()
    return _NC


def _in_maps(x: np.ndarray) -> list:
    import ml_dtypes

    bf16 = ml_dtypes.bfloat16
    x = np.asarray(x)
    xb = np.ascontiguousarray(x, dtype=np.float32).astype(bf16)
    H128 = _hadamard(128)
    s4 = np.ascontiguousarray(
        np.concatenate([H128, H128, H128, -H128], axis=1)
    ).astype(bf16)
    H = _hadamard(N)
    hs = np.ascontiguousarray(
        (H / np.float32(512.0))
        .reshape(4, 128, N)
        .transpose(1, 0, 2)
        .reshape(128, 4 * N)
    ).astype(bf16)
    return [
        {
            "x": np.ascontiguousarray(
                xb[i * BPC : (i + 1) * BPC].reshape(BPC, 128, 4 * D)
            ),
            "s4": s4,
            "hs": hs,
        }
        for i in range(NCORES)
    ]


def kernel(x: np.ndarray) -> np.ndarray:
    nc = _get_nc()
    res = run_bass_kernel_spmd(nc, _in_maps(x), list(range(NCORES))).results
    return np.concatenate(
        [np.asarray(r["y"]).reshape(BPC, N, D).astype(np.float32) for r in res],
        axis=0,
    )


# revision 9
# speedup vs baseline: 5.2777x; 1.2170x over previous
import sys

import numpy as np

for p in ("/opt/trn_rl_repo",):
    if p not in sys.path:
        sys.path.insert(0, p)

import concourse.bass as bass  # noqa: E402
import concourse.tile as tile  # noqa: E402
from concourse import bacc, mybir  # noqa: E402
from concourse.bass_utils import run_bass_kernel_spmd  # noqa: E402

B, N, D = 128, 512, 512
NCORES = 8
BPC = B // NCORES  # 16 batch items per core
F32 = mybir.dt.float32
BF16 = mybir.dt.bfloat16


def _hadamard(n: int) -> np.ndarray:
    H = np.array([[1.0]], dtype=np.float32)
    base = np.array([[1.0, 1.0], [1.0, -1.0]], dtype=np.float32)
    while H.shape[0] < n:
        H = np.kron(H, base)
    return H


def _build():
    # Row m = 4q+j lives on partition q, free block j (4 consecutive DRAM
    # rows per partition -> one 512KB DMA per batch with 4KB lines).
    # H512[4q+j, 4p+i] = H128[q,p]*H4[j,i]   (H512 = H128 (x) H4)
    # H512[dc*128+r, f*128+s] = H4[dc,f]*H128[r,s]  (H512 = H4 (x) H128)
    # Both transforms fold one H2 level into PSUM accumulation via paired
    # moving operands [S|S] / [S|-S] (N=256 matmuls); the remaining H2
    # level is one add + one sub on [128,1024] tiles (DVE, 2x rate).
    # Output leaves in (half, j, f', s) column order; host unpermutes.
    nc = bacc.Bacc("TRN2", target_bir_lowering=False, debug=False)
    x_d = nc.dram_tensor("x", [BPC, 128, 4 * D], BF16, kind="ExternalInput").ap()
    s4_d = nc.dram_tensor("s4", [128, 512], BF16, kind="ExternalInput").ap()
    s4b_d = nc.dram_tensor("s4b", [128, 512], BF16, kind="ExternalInput").ap()
    y_d = nc.dram_tensor("y", [BPC, 128, 4 * D], BF16, kind="ExternalOutput").ap()

    with tile.TileContext(nc) as tc:
        with (
            tc.tile_pool(name="const", bufs=1) as const_pool,
            tc.tile_pool(name="xp", bufs=3) as x_pool,
            tc.tile_pool(name="cs", bufs=2) as cs_pool,
            tc.tile_pool(name="csb", bufs=2) as csb_pool,
            tc.tile_pool(name="tp", bufs=2) as t_pool,
            tc.tile_pool(name="yp", bufs=2) as y_pool,
            tc.tile_pool(name="psa", bufs=1, space="PSUM") as psum_a,
            tc.tile_pool(name="psb", bufs=1, space="PSUM") as psum_b,
        ):
            s4_sb = const_pool.tile([128, 512], BF16, tag="s4")
            nc.sync.dma_start(s4_sb[:], s4_d[:])
            s4b_sb = const_pool.tile([128, 512], BF16, tag="s4b")
            nc.sync.dma_start(s4b_sb[:], s4b_d[:])

            sp2 = s4_sb[:, 0:256]  # [S | S]
            sn2 = s4_sb[:, 256:512]  # [S | -S]
            sp2b = s4b_sb[:, 0:256]  # [S | S] / 512
            sn2b = s4b_sb[:, 256:512]  # [S | -S] / 512

            state = None  # pending (ttP, ttM, b) from previous batch

            def emit_B(ttP, ttM, bprev):
                # B: y[4p+j, f*128+s] = sum_dc H4[dc,f] v_dc[4p+j, s]
                # v_dc[n, s] = sum_r t_T[dc*128+r, n] * H128[r,s]/512
                psB01 = psum_b.tile([128, 1024], F32, tag="b01")
                psB23 = psum_b.tile([128, 1024], F32, tag="b23")
                for j in range(4):
                    o = slice(j * 256, j * 256 + 256)
                    src = ttP if j < 2 else ttM
                    jj = (j % 2) * 128
                    t0 = src[:, 0 * 256 + jj : 0 * 256 + jj + 128]
                    t1 = src[:, 1 * 256 + jj : 1 * 256 + jj + 128]
                    t2 = src[:, 2 * 256 + jj : 2 * 256 + jj + 128]
                    t3 = src[:, 3 * 256 + jj : 3 * 256 + jj + 128]
                    nc.tensor.matmul(psB01[:, o], t0, sp2b, start=True, stop=False)
                    nc.tensor.matmul(psB01[:, o], t1, sn2b, start=False, stop=True)
                    nc.tensor.matmul(psB23[:, o], t2, sp2b, start=True, stop=False)
                    nc.tensor.matmul(psB23[:, o], t3, sn2b, start=False, stop=True)
                csB01 = csb_pool.tile([128, 1024], BF16, tag="b01")
                csB23 = csb_pool.tile([128, 1024], BF16, tag="b23")
                nc.scalar.copy(csB01[:, 0:512], psB01[:, 0:512])
                nc.scalar.copy(csB01[:, 512:1024], psB01[:, 512:1024])
                nc.vector.tensor_copy(csB23[:], psB23[:])
                yP = y_pool.tile([128, 1024], BF16, tag="yP")
                yM = y_pool.tile([128, 1024], BF16, tag="yM")
                nc.vector.tensor_add(yP[:], csB01[:], csB23[:])
                nc.vector.tensor_sub(yM[:], csB01[:], csB23[:])
                nc.sync.dma_start(y_d[bprev][:, 0:1024], yP[:])
                nc.sync.dma_start(y_d[bprev][:, 1024:2048], yM[:])

            for b in range(BPC):
                xb = x_pool.tile([128, 4 * D], BF16)
                nc.sync.dma_start(xb[:], x_d[b])

                # A: u_j[dd, p] = sum_q x[4q+j, dt*128+dd] * H128[q, p]
                # psum free layout dt*256 + (s:0-127 | d:128-255)
                ps_sd01 = psum_a.tile([128, 1024], F32, tag="sd01")
                ps_sd23 = psum_a.tile([128, 1024], F32, tag="sd23")
                for dt in range(4):
                    o = slice(dt * 256, dt * 256 + 256)
                    x0 = xb[:, 0 * D + dt * 128 : 0 * D + dt * 128 + 128]
                    x1 = xb[:, 1 * D + dt * 128 : 1 * D + dt * 128 + 128]
                    x2 = xb[:, 2 * D + dt * 128 : 2 * D + dt * 128 + 128]
                    x3 = xb[:, 3 * D + dt * 128 : 3 * D + dt * 128 + 128]
                    nc.tensor.matmul(ps_sd01[:, o], x0, sp2, start=True, stop=False)
                    nc.tensor.matmul(ps_sd01[:, o], x1, sn2, start=False, stop=True)
                    nc.tensor.matmul(ps_sd23[:, o], x2, sp2, start=True, stop=False)
                    nc.tensor.matmul(ps_sd23[:, o], x3, sn2, start=False, stop=True)
                cs_sd01 = cs_pool.tile([128, 1024], BF16, tag="sd01")
                cs_sd23 = cs_pool.tile([128, 1024], BF16, tag="sd23")
                nc.scalar.copy(cs_sd01[:, 0:512], ps_sd01[:, 0:512])
                nc.scalar.copy(cs_sd01[:, 512:1024], ps_sd01[:, 512:1024])
                nc.scalar.copy(cs_sd23[:, 0:512], ps_sd23[:, 0:512])
                nc.scalar.copy(cs_sd23[:, 512:1024], ps_sd23[:, 512:1024])
                # Remaining H2 level: ttP = [t0|t1], ttM = [t2|t3]
                # layout dt*256 + i'*128 + p
                ttP = t_pool.tile([128, 1024], BF16, tag="P")
                ttM = t_pool.tile([128, 1024], BF16, tag="M")
                nc.vector.tensor_add(ttP[:], cs_sd01[:], cs_sd23[:])
                nc.vector.tensor_sub(ttM[:], cs_sd01[:], cs_sd23[:])
                if state is not None:
                    emit_B(*state)
                state = (ttP, ttM, b)

            emit_B(*state)

    nc.compile()
    return nc


_NC = None


def _get_nc():
    global _NC
    if _NC is None:
        _NC = _build()
    return _NC


def _in_maps(x: np.ndarray) -> list:
    import ml_dtypes

    bf16 = ml_dtypes.bfloat16
    x = np.asarray(x)
    xb = np.ascontiguousarray(x, dtype=np.float32).astype(bf16)
    H128 = _hadamard(128)
    s4 = np.ascontiguousarray(
        np.concatenate([H128, H128, H128, -H128], axis=1)
    ).astype(bf16)
    s4b = np.ascontiguousarray(s4.astype(np.float32) / np.float32(512.0)).astype(
        bf16
    )
    return [
        {
            "x": np.ascontiguousarray(
                xb[i * BPC : (i + 1) * BPC].reshape(BPC, 128, 4 * D)
            ),
            "s4": s4,
            "s4b": s4b,
        }
        for i in range(NCORES)
    ]


def kernel(x: np.ndarray) -> np.ndarray:
    nc = _get_nc()
    res = run_bass_kernel_spmd(nc, _in_maps(x), list(range(NCORES))).results
    outs = []
    for r in res:
        # y cols: half(P/M)*1024 + j*256 + f2*128 + s ; rows: 4p+j on
        # partition p -> y[4p+j, (half*2+f2)*128+s]
        o = np.asarray(r["y"]).reshape(BPC, 128, 2, 4, 2, 128)
        o = o.transpose(0, 1, 3, 2, 4, 5).reshape(BPC, N, D)
        outs.append(o.astype(np.float32))
    return np.concatenate(outs, axis=0)


# revision 13
# speedup vs baseline: 5.4432x; 1.0314x over previous
import sys

import numpy as np

for p in ("/opt/trn_rl_repo",):
    if p not in sys.path:
        sys.path.insert(0, p)

import concourse.bass as bass  # noqa: E402
import concourse.tile as tile  # noqa: E402
from concourse import bacc, mybir  # noqa: E402
from concourse.bass_utils import run_bass_kernel_spmd  # noqa: E402

B, N, D = 128, 512, 512
NCORES = 8
BPC = B // NCORES  # 16 batch items per core
F32 = mybir.dt.float32
BF16 = mybir.dt.bfloat16


def _hadamard(n: int) -> np.ndarray:
    H = np.array([[1.0]], dtype=np.float32)
    base = np.array([[1.0, 1.0], [1.0, -1.0]], dtype=np.float32)
    while H.shape[0] < n:
        H = np.kron(H, base)
    return H


def _build():
    # Row m = 4q+j lives on partition q, free block j (4 consecutive DRAM
    # rows per partition -> one 512KB DMA per batch with 4KB lines).
    # H512[4q+j, 4p+i] = H128[q,p]*H4[j,i]   (H512 = H128 (x) H4)
    # H512[dc*128+r, f*128+s] = H4[dc,f]*H128[r,s]  (H512 = H4 (x) H128)
    # Both transforms fold one H2 level into PSUM accumulation via paired
    # moving operands [S|S] / [S|-S] (N=256 matmuls); the remaining H2
    # level is one add + one sub on [128,1024] tiles (DVE, 2x rate).
    # Output leaves in (half, j, f', s) column order; host unpermutes.
    nc = bacc.Bacc("TRN2", target_bir_lowering=False, debug=False)
    x_d = nc.dram_tensor("x", [BPC, 128, 4 * D], BF16, kind="ExternalInput").ap()
    s4_d = nc.dram_tensor("s4", [128, 512], BF16, kind="ExternalInput").ap()
    s4b_d = nc.dram_tensor("s4b", [128, 512], BF16, kind="ExternalInput").ap()
    y_d = nc.dram_tensor("y", [BPC, 128, 4 * D], BF16, kind="ExternalOutput").ap()

    with tile.TileContext(nc) as tc:
        with (
            tc.tile_pool(name="const", bufs=1) as const_pool,
            tc.tile_pool(name="xp", bufs=3) as x_pool,
            tc.tile_pool(name="cs", bufs=2) as cs_pool,
            tc.tile_pool(name="csb", bufs=2) as csb_pool,
            tc.tile_pool(name="tp", bufs=2) as t_pool,
            tc.tile_pool(name="yp", bufs=2) as y_pool,
            tc.tile_pool(name="psa", bufs=1, space="PSUM") as psum_a,
            tc.tile_pool(name="psb", bufs=1, space="PSUM") as psum_b,
        ):
            s4_sb = const_pool.tile([128, 512], BF16, tag="s4")
            s4b_sb = const_pool.tile([128, 512], BF16, tag="s4b")

            sp2 = s4_sb[:, 0:256]  # [S | S]
            sn2 = s4_sb[:, 256:512]  # [S | -S]
            sp2b = s4b_sb[:, 0:256]  # [S | S] / 512
            sn2b = s4b_sb[:, 256:512]  # [S | -S] / 512

            state = None  # pending (ttP, ttM, b) from previous batch

            def emit_B(ttP, ttM, bprev):
                # B: y[4p+j, f*128+s] = sum_dc H4[dc,f] v_dc[4p+j, s]
                # v_dc[n, s] = sum_r t_T[dc*128+r, n] * H128[r,s]/512
                psB01 = psum_b.tile([128, 1024], F32, tag="b01")
                psB23 = psum_b.tile([128, 1024], F32, tag="b23")
                for j in range(4):
                    o = slice(j * 256, j * 256 + 256)
                    src = ttP if j < 2 else ttM
                    jj = (j % 2) * 128
                    t0 = src[:, 0 * 256 + jj : 0 * 256 + jj + 128]
                    t1 = src[:, 1 * 256 + jj : 1 * 256 + jj + 128]
                    t2 = src[:, 2 * 256 + jj : 2 * 256 + jj + 128]
                    t3 = src[:, 3 * 256 + jj : 3 * 256 + jj + 128]
                    nc.tensor.matmul(psB01[:, o], t0, sp2b, start=True, stop=False)
                    nc.tensor.matmul(psB01[:, o], t1, sn2b, start=False, stop=True)
                    nc.tensor.matmul(psB23[:, o], t2, sp2b, start=True, stop=False)
                    nc.tensor.matmul(psB23[:, o], t3, sn2b, start=False, stop=True)
                csB01 = csb_pool.tile([128, 1024], BF16, tag="b01")
                csB23 = csb_pool.tile([128, 1024], BF16, tag="b23")
                nc.scalar.copy(csB01[:], psB01[:])
                nc.vector.tensor_copy(csB23[:], psB23[:])
                yPM = y_pool.tile([128, 2048], BF16, tag="yPM")
                nc.vector.tensor_add(yPM[:, 0:1024], csB01[:], csB23[:])
                nc.vector.tensor_sub(yPM[:, 1024:2048], csB01[:], csB23[:])
                nc.sync.dma_start(y_d[bprev], yPM[:])

            for b in range(BPC):
                xb = x_pool.tile([128, 4 * D], BF16)
                nc.sync.dma_start(xb[:], x_d[b])
                if b == 0:
                    nc.sync.dma_start(s4_sb[:], s4_d[:])
                    nc.sync.dma_start(s4b_sb[:], s4b_d[:])

                # A: u_j[dd, p] = sum_q x[4q+j, dt*128+dd] * H128[q, p]
                # psum free layout dt*256 + (s:0-127 | d:128-255)
                ps_sd01 = psum_a.tile([128, 1024], F32, tag="sd01")
                ps_sd23 = psum_a.tile([128, 1024], F32, tag="sd23")
                for dt in range(4):
                    o = slice(dt * 256, dt * 256 + 256)
                    x0 = xb[:, 0 * D + dt * 128 : 0 * D + dt * 128 + 128]
                    x1 = xb[:, 1 * D + dt * 128 : 1 * D + dt * 128 + 128]
                    x2 = xb[:, 2 * D + dt * 128 : 2 * D + dt * 128 + 128]
                    x3 = xb[:, 3 * D + dt * 128 : 3 * D + dt * 128 + 128]
                    nc.tensor.matmul(ps_sd01[:, o], x0, sp2, start=True, stop=False)
                    nc.tensor.matmul(ps_sd01[:, o], x1, sn2, start=False, stop=True)
                    nc.tensor.matmul(ps_sd23[:, o], x2, sp2, start=True, stop=False)
                    nc.tensor.matmul(ps_sd23[:, o], x3, sn2, start=False, stop=True)
                cs_sd01 = cs_pool.tile([128, 1024], BF16, tag="sd01")
                cs_sd23 = cs_pool.tile([128, 1024], BF16, tag="sd23")
                nc.scalar.copy(cs_sd01[:], ps_sd01[:])
                nc.scalar.copy(cs_sd23[:], ps_sd23[:])
                # Remaining H2 level: ttP = [t0|t1], ttM = [t2|t3]
                # layout dt*256 + i'*128 + p
                ttP = t_pool.tile([128, 1024], BF16, tag="P")
                ttM = t_pool.tile([128, 1024], BF16, tag="M")
                nc.vector.tensor_add(ttP[:], cs_sd01[:], cs_sd23[:])
                nc.vector.tensor_sub(ttM[:], cs_sd01[:], cs_sd23[:])
                if state is not None:
                    emit_B(*state)
                state = (ttP, ttM, b)

            emit_B(*state)

    nc.compile()
    return nc


_NC = None


def _get_nc():
    global _NC
    if _NC is None:
        _NC = _build()
    return _NC


def _in_maps(x: np.ndarray) -> list:
    import ml_dtypes

    bf16 = ml_dtypes.bfloat16
    x = np.asarray(x)
    xb = np.ascontiguousarray(x, dtype=np.float32).astype(bf16)
    H128 = _hadamard(128)
    s4 = np.ascontiguousarray(
        np.concatenate([H128, H128, H128, -H128], axis=1)
    ).astype(bf16)
    s4b = np.ascontiguousarray(s4.astype(np.float32) / np.float32(512.0)).astype(
        bf16
    )
    return [
        {
            "x": np.ascontiguousarray(
                xb[i * BPC : (i + 1) * BPC].reshape(BPC, 128, 4 * D)
            ),
            "s4": s4,
            "s4b": s4b,
        }
        for i in range(NCORES)
    ]


def kernel(x: np.ndarray) -> np.ndarray:
    nc = _get_nc()
    res = run_bass_kernel_spmd(nc, _in_maps(x), list(range(NCORES))).results
    outs = []
    for r in res:
        # y cols: half(P/M)*1024 + j*256 + f2*128 + s ; rows: 4p+j on
        # partition p -> y[4p+j, (half*2+f2)*128+s]
        o = np.asarray(r["y"]).reshape(BPC, 128, 2, 4, 2, 128)
        o = o.transpose(0, 1, 3, 2, 4, 5).reshape(BPC, N, D)
        outs.append(o.astype(np.float32))
    return np.concatenate(outs, axis=0)
